# revision 1
# baseline (speedup 1.0000x reference)
"""GQA kernel for Trainium2, 8 NeuronCores.

Problem: x[2,2048,2048] -> GQA(16 heads, 4 kv groups, dk=128) -> out[2,2048,2048]

Sharding: core c handles (batch b = c//4, kv-group g = c%4), i.e. the 4 query
heads of one group on one batch. Zero replication of FLOPs across cores:
per-core work = Qproj(4 heads) + K/Vproj(1 group) + attention(4 heads) +
row-slice of the O projection. Host sums the 4 per-group partial outputs
per batch (the row-parallel O-proj reduction) and adds bo.

On-core dataflow (all matmuls contract over the partition dim):
  xT [D,S] (host-transposed)  --PE-->  QT [dk,S] per head, KT [dk,S], VT [dk,S]
  scoresT[sk,sq] = KT_chunk.T @ QT      (f32r, full-rate)
  attnT = exp(scoresT / sqrt(dk))       (ACT, bf16 out)
  attn@[V|1] via lhsT=attnT chunks      (bf16; extra ones-column of V gives the
                                         softmax denominator for free)
  normalize rows (DVE), PE-transpose back to [dk,sq], O-proj vs Wo rows (f32r)
"""

import math

import numpy as np

import concourse.bass as bass
import concourse.mybir as mybir
import concourse.tile as tile
from concourse import bacc
from concourse.bass_utils import run_bass_kernel_spmd
from concourse.masks import make_identity

F32 = mybir.dt.float32
F32R = mybir.dt.float32r
BF16 = mybir.dt.bfloat16

D = 2048          # d_model
S = 2048          # seq len
DK = 128          # head dim
HPG = 4           # heads per kv group
QCOLS = HPG * DK  # 512 q columns per core
N_CORES = 8
SCALE = 1.0 / math.sqrt(DK)

SJ = 256                    # seq chunk (free dim of proj/scores matmuls)
NJ = S // SJ                # 8 chunks
NSK = S // 128              # 16 key chunks
ND = D // 128               # 16 d_model chunks


def _r(ap):
    return ap.bitcast(F32R)


def build_program(n_reps=1):
    nc = bacc.Bacc("TRN2", target_bir_lowering=False, debug=False,
                   num_devices=N_CORES)

    xt = nc.dram_tensor("xt", [D, S], F32, kind="ExternalInput").ap()
    wq = nc.dram_tensor("wq", [D, QCOLS], F32, kind="ExternalInput").ap()
    wk = nc.dram_tensor("wk", [D, DK], F32, kind="ExternalInput").ap()
    wv = nc.dram_tensor("wv", [D, DK], F32, kind="ExternalInput").ap()
    wo = nc.dram_tensor("wo", [QCOLS, D], F32, kind="ExternalInput").ap()
    bq = nc.dram_tensor("bq", [QCOLS], F32, kind="ExternalInput").ap()
    bk = nc.dram_tensor("bk", [DK], F32, kind="ExternalInput").ap()
    bv = nc.dram_tensor("bv", [DK], F32, kind="ExternalInput").ap()
    out = nc.dram_tensor("out", [S, D], F32, kind="ExternalOutput").ap()

    with tile.TileContext(nc) as tc:
      for _rep in range(n_reps):
        with (
            tc.tile_pool(name="singles", bufs=1) as singles,
            tc.tile_pool(name="xp", bufs=3) as xpool,
            tc.tile_pool(name="attn", bufs=24) as attnpool,
            tc.tile_pool(name="aot", bufs=2) as aotpool,
            tc.tile_pool(name="osb", bufs=4) as outpool,
            tc.tile_pool(name="small", bufs=4) as smallpool,
            tc.tile_pool(name="psA", bufs=3, space="PSUM") as psA,
            tc.tile_pool(name="psAV", bufs=3, space="PSUM") as psAV,
            tc.tile_pool(name="psO", bufs=2, space="PSUM") as psO,
        ):
            # ---- resident weights / biases ----
            wq_sb = singles.tile([128, ND, QCOLS], F32R)
            nc.sync.dma_start(out=wq_sb, in_=wq.rearrange("(c p) n -> p c n", p=128).bitcast(F32R))
            wk_sb = singles.tile([128, ND, DK], F32R)
            nc.sync.dma_start(out=wk_sb, in_=wk.rearrange("(c p) n -> p c n", p=128).bitcast(F32R))
            wv_sb = singles.tile([128, ND, DK], F32R)
            nc.sync.dma_start(out=wv_sb, in_=wv.rearrange("(c p) n -> p c n", p=128).bitcast(F32R))
            wo_sb = singles.tile([128, HPG, D], F32R)
            nc.sync.dma_start(out=wo_sb, in_=wo.rearrange("(h p) n -> p h n", p=128).bitcast(F32R))
            bq_sb = singles.tile([128, HPG], F32)
            nc.sync.dma_start(out=bq_sb, in_=bq.rearrange("(h p) -> p h", p=128))
            bk_sb = singles.tile([128, 1], F32)
            nc.sync.dma_start(out=bk_sb, in_=bk.unsqueeze(1))
            bv_sb = singles.tile([128, 1], F32)
            nc.sync.dma_start(out=bv_sb, in_=bv.unsqueeze(1))

            ident32 = singles.tile([128, 128], F32)
            make_identity(nc, ident32)
            ident16 = singles.tile([128, 128], BF16)
            make_identity(nc, ident16)

            qt_sb = singles.tile([128, HPG, S], F32R)    # QT per head [dk, S]
            kt_sb = singles.tile([128, S], F32R)         # KT [dk, S]
            vt_sb = singles.tile([128, S], BF16)        # VT [dk, S]
            vones = singles.tile([128, NSK, 132], BF16)  # [V | 1] per key chunk
            nc.vector.memset(vones[:, :, 128:129], 1.0)

            # ---- phase B: projections (stream xT chunks) ----
            for j in range(NJ):
                sl = bass.ts(j, SJ)
                xt_sb = xpool.tile([128, ND, SJ], F32R)
                nc.sync.dma_start(
                    out=xt_sb, in_=xt[:, sl].rearrange("(c p) s -> p c s", p=128).bitcast(F32R))
                for h in range(HPG):
                    pq = psA.tile([128, SJ], F32, tag="big")
                    for d in range(ND):
                        nc.tensor.matmul(
                            pq, lhsT=wq_sb[:, d, bass.ts(h, 128)],
                            rhs=xt_sb[:, d, :],
                            start=(d == 0), stop=(d == ND - 1))
                    nc.scalar.activation(
                        out=qt_sb[:, h, sl], in_=pq,
                        func=mybir.ActivationFunctionType.Identity,
                        bias=bq_sb[:, h:h + 1])
                pk = psA.tile([128, SJ], F32, tag="big")
                for d in range(ND):
                    nc.tensor.matmul(pk, lhsT=wk_sb[:, d, :],
                                     rhs=xt_sb[:, d, :],
                                     start=(d == 0), stop=(d == ND - 1))
                nc.scalar.activation(out=kt_sb[:, sl], in_=pk,
                                     func=mybir.ActivationFunctionType.Identity,
                                     bias=bk_sb)
                pv = psA.tile([128, SJ], F32, tag="big")
                for d in range(ND):
                    nc.tensor.matmul(pv, lhsT=wv_sb[:, d, :],
                                     rhs=xt_sb[:, d, :],
                                     start=(d == 0), stop=(d == ND - 1))
                nc.scalar.activation(out=vt_sb[:, sl], in_=pv,
                                     func=mybir.ActivationFunctionType.Identity,
                                     bias=bv_sb)

            # VT [dk,S] -> V [S,dk] chunks with a ones column appended
            for sk in range(NSK):
                pt = psAV.tile([128, 128], BF16, tag="av")
                nc.tensor.transpose(pt, vt_sb[:, bass.ts(sk, 128)], ident16)
                nc.vector.tensor_copy(vones[:, sk, 0:128], pt)

            # ---- phase C: attention + O-projection, per 256-wide q block ----
            for j in range(NJ):
                sl = bass.ts(j, SJ)
                aot = aotpool.tile([128, HPG, SJ], F32R)  # attn-out.T [dk, sq]
                for h in range(HPG):
                    attns = []
                    for sk in range(NSK):
                        ps = psA.tile([128, SJ], F32, tag="big")
                        nc.tensor.matmul(ps, lhsT=kt_sb[:, bass.ts(sk, 128)],
                                         rhs=qt_sb[:, h, sl],
                                         start=True, stop=True)
                        a = attnpool.tile([128, SJ], BF16)
                        nc.scalar.activation(
                            out=a, in_=ps,
                            func=mybir.ActivationFunctionType.Exp, scale=SCALE)
                        attns.append(a)
                    for sub in range(SJ // 128):
                        pav = psAV.tile([128, 132], F32, tag="av")
                        for sk in range(NSK):
                            nc.tensor.matmul(
                                pav[:, 0:129],
                                lhsT=attns[sk][:, bass.ts(sub, 128)],
                                rhs=vones[:, sk, 0:129],
                                start=(sk == 0), stop=(sk == NSK - 1))
                        recip = smallpool.tile([128, 1], F32)
                        nc.vector.reciprocal(recip, pav[:, 128:129])
                        ao = smallpool.tile([128, 128], F32, tag="ao")
                        nc.vector.tensor_scalar_mul(ao, pav[:, 0:128], recip)
                        pt = psAV.tile([128, 128], F32, tag="av")
                        nc.tensor.transpose(pt, ao, ident32)
                        nc.vector.tensor_copy(aot[:, h, bass.ts(sub, 128)], pt)
                # O projection for q rows [j*SJ, (j+1)*SJ)
                for sub in range(SJ // 128):
                    for dc in range(D // 512):
                        po = psO.tile([128, 512], F32, tag="o")
                        for h in range(HPG):
                            nc.tensor.matmul(
                                po, lhsT=aot[:, h, bass.ts(sub, 128)],
                                rhs=wo_sb[:, h, bass.ts(dc, 512)],
                                start=(h == 0), stop=(h == HPG - 1))
                        osb = outpool.tile([128, 512], F32)
                        nc.vector.tensor_copy(osb, po)
                        nc.sync.dma_start(
                            out=out[j * SJ + sub * 128: j * SJ + (sub + 1) * 128,
                                    bass.ts(dc, 512)],
                            in_=osb)

    nc.compile()
    return nc


_NC_CACHE = None


def _get_program():
    global _NC_CACHE
    if _NC_CACHE is None:
        _NC_CACHE = build_program()
    return _NC_CACHE


def kernel(x, Wq, bq, Wk, bk, Wv, bv, Wo, bo):
    x = np.asarray(x, np.float32)
    nc = _get_program()

    in_maps = []
    xts = [np.ascontiguousarray(x[b].T) for b in range(x.shape[0])]
    for c in range(N_CORES):
        b, g = divmod(c, HPG)
        in_maps.append({
            "xt": xts[b],
            "wq": np.ascontiguousarray(np.asarray(Wq, np.float32)[:, g * QCOLS:(g + 1) * QCOLS]),
            "wk": np.ascontiguousarray(np.asarray(Wk, np.float32)[:, g * DK:(g + 1) * DK]),
            "wv": np.ascontiguousarray(np.asarray(Wv, np.float32)[:, g * DK:(g + 1) * DK]),
            "wo": np.ascontiguousarray(np.asarray(Wo, np.float32)[g * QCOLS:(g + 1) * QCOLS, :]),
            "bq": np.ascontiguousarray(np.asarray(bq, np.float32)[g * QCOLS:(g + 1) * QCOLS]),
            "bk": np.ascontiguousarray(np.asarray(bk, np.float32)[g * DK:(g + 1) * DK]),
            "bv": np.ascontiguousarray(np.asarray(bv, np.float32)[g * DK:(g + 1) * DK]),
        })

    res = run_bass_kernel_spmd(nc, in_maps, core_ids=list(range(N_CORES))).results

    outv = np.zeros((x.shape[0], S, D), np.float32)
    for c in range(N_CORES):
        b = c // HPG
        outv[b] += res[c]["out"]
    outv += np.asarray(bo, np.float32)
    return outv



# revision 3
# speedup vs baseline: 1.2574x; 1.2574x over previous
"""GQA kernel for Trainium2, 8 NeuronCores.

Problem: x[2,2048,2048] -> GQA(16 heads, 4 kv groups, dk=128) -> out[2,2048,2048]

Sharding: core c handles (batch b = c//4, kv-group g = c%4), i.e. the 4 query
heads of one group on one batch. Zero replication of FLOPs across cores.
Host sums the 4 per-group partial outputs per batch (row-parallel O-proj
reduction) and adds bo.

All matmul operands are bf16 (host-converted), PSUM accumulation stays f32:
  xT [D,S] bf16 (host-transposed)  --PE-->  QT [dk,S]/head, KT [dk,S] (bf16)
  V is projected directly in [t, dk] orientation (lhsT = xT chunk), so no
    V transpose is needed; a ones-column next to V gives softmax denominators.
  scoresT[sk,sq] = KT_chunk.T @ QT   (bf16, 512-wide)
  exp on ACT in [128, 2x512] chunks from 2 PSUM banks (amortizes overhead);
    biases are applied on DVE so ACT does nothing but exp.
  attn@[V|1] accumulates over the 16 key chunks; DVE normalizes by the
    denominator column; PE transposes attn-out back to [dk, sq] (bf16);
    row-parallel O-projection accumulates the 4 heads into PSUM.
"""

import math

import numpy as np
import ml_dtypes

import concourse.bass as bass
import concourse.mybir as mybir
import concourse.tile as tile
from concourse import bacc
from concourse.bass_utils import run_bass_kernel_spmd
from concourse.masks import make_identity

F32 = mybir.dt.float32
BF16 = mybir.dt.bfloat16

D = 2048          # d_model
S = 2048          # seq len
DK = 128          # head dim
HPG = 4           # heads per kv group
QCOLS = HPG * DK  # 512 q columns per core
N_CORES = 8
SCALE = 1.0 / math.sqrt(DK)

SJ = 512                    # seq chunk (free dim of proj/scores matmuls)
NJ = S // SJ                # 4 chunks
NSK = S // 128              # 16 key chunks
ND = D // 128               # 16 d_model chunks
GRP = 2                     # key chunks per exp group (2 PSUM banks)
NG = NSK // GRP             # 8 exp groups
PAV_W = 196                 # AV psum tile width: [0:129]=AV|denom, f32[132:196]
                            # bitcast to bf16 [128] = transpose scratch


def build_program(n_reps=1):
    nc = bacc.Bacc("TRN2", target_bir_lowering=False, debug=False,
                   num_devices=N_CORES)

    xt = nc.dram_tensor("xt", [D, S], BF16, kind="ExternalInput").ap()
    wq = nc.dram_tensor("wq", [D, QCOLS], BF16, kind="ExternalInput").ap()
    wk = nc.dram_tensor("wk", [D, DK], BF16, kind="ExternalInput").ap()
    wv = nc.dram_tensor("wv", [D, DK], BF16, kind="ExternalInput").ap()
    wo = nc.dram_tensor("wo", [QCOLS, D], BF16, kind="ExternalInput").ap()
    bq = nc.dram_tensor("bq", [QCOLS], F32, kind="ExternalInput").ap()
    bk = nc.dram_tensor("bk", [DK], F32, kind="ExternalInput").ap()
    bvb = nc.dram_tensor("bvb", [DK, DK], F32, kind="ExternalInput").ap()
    out = nc.dram_tensor("out", [S, D], F32, kind="ExternalOutput").ap()

    with tile.TileContext(nc) as tc:
      for _rep in range(n_reps):
        with (
            tc.tile_pool(name="singles", bufs=1) as singles,
            tc.tile_pool(name="xp", bufs=2) as xpool,
            tc.tile_pool(name="attn", bufs=16) as attnpool,
            tc.tile_pool(name="aot", bufs=2) as aotpool,
            tc.tile_pool(name="osb", bufs=2) as outpool,
            tc.tile_pool(name="small", bufs=6) as smallpool,
            tc.tile_pool(name="psS", bufs=2, space="PSUM") as psS,
            tc.tile_pool(name="psAV", bufs=2, space="PSUM") as psAV,
            tc.tile_pool(name="psO", bufs=2, space="PSUM") as psO,
        ):
            # ---- resident weights / biases; DMA order = lead-in order ----
            wk_sb = singles.tile([128, ND, DK], BF16)
            nc.sync.dma_start(out=wk_sb, in_=wk.rearrange("(c p) n -> p c n", p=128))
            bk_sb = singles.tile([128, 1], F32)
            nc.sync.dma_start(out=bk_sb, in_=bk.unsqueeze(1))
            wv_sb = singles.tile([128, ND, DK], BF16)
            nc.sync.dma_start(out=wv_sb, in_=wv.rearrange("(c p) n -> p c n", p=128))
            bvb_sb = singles.tile([128, DK], F32)
            nc.sync.dma_start(out=bvb_sb, in_=bvb)

            ident16 = singles.tile([128, 128], BF16)
            make_identity(nc, ident16)

            qt_sb = singles.tile([128, HPG, S], BF16)    # QT per head [dk, S]
            kt_sb = singles.tile([128, S], BF16)         # KT [dk, S]
            vones = singles.tile([128, NSK, 132], BF16)  # [V | 1] per key chunk
            nc.vector.memset(vones[:, :, 128:129], 1.0)

            wq_sb = singles.tile([128, ND, QCOLS], BF16)
            bq_sb = singles.tile([128, HPG], F32)
            wo_sb = singles.tile([128, HPG, D], BF16)

            # ---- phase B: projections (stream xT chunks) ----
            for j in range(NJ):
                sl = bass.ts(j, SJ)
                xt_sb = xpool.tile([128, ND, SJ], BF16)
                xsrc = xt[:, sl].rearrange("(c p) s -> p c s", p=128)
                for q in range(4):
                    nc.sync.dma_start(out=xt_sb[:, 4 * q:4 * q + 4, :],
                                      in_=xsrc[:, 4 * q:4 * q + 4, :])
                if j == 0:
                    # late weights stream behind the first x chunk
                    nc.sync.dma_start(
                        out=wq_sb, in_=wq.rearrange("(c p) n -> p c n", p=128))
                    nc.sync.dma_start(
                        out=bq_sb, in_=bq.rearrange("(h p) -> p h", p=128))
                    nc.sync.dma_start(
                        out=wo_sb, in_=wo.rearrange("(h p) n -> p h n", p=128))
                pk = psO.tile([128, SJ], F32, tag="o")
                for d in range(ND):
                    nc.tensor.matmul(pk, lhsT=wk_sb[:, d, :],
                                     rhs=xt_sb[:, d, :],
                                     start=(d == 0), stop=(d == ND - 1))
                nc.vector.tensor_scalar_add(kt_sb[:, sl], pk, bk_sb)
                for sub in range(SJ // 128):
                    pv = psAV.tile([128, PAV_W], F32, tag="av")
                    tsl = bass.ts(sub, 128)
                    for d in range(ND):
                        nc.tensor.matmul(pv[:, 0:128], lhsT=xt_sb[:, d, tsl],
                                         rhs=wv_sb[:, d, :],
                                         start=(d == 0), stop=(d == ND - 1))
                    nc.vector.tensor_add(vones[:, 4 * j + sub, 0:128],
                                         pv[:, 0:128], bvb_sb)
                for h in range(HPG):
                    pq = psO.tile([128, SJ], F32, tag="o")
                    for d in range(ND):
                        nc.tensor.matmul(
                            pq, lhsT=wq_sb[:, d, bass.ts(h, 128)],
                            rhs=xt_sb[:, d, :],
                            start=(d == 0), stop=(d == ND - 1))
                    nc.vector.tensor_scalar_add(qt_sb[:, h, sl], pq,
                                                bq_sb[:, h:h + 1])

            # ---- phase C: attention + O-projection, per 512-wide q block ----
            for j in range(NJ):
                sl = bass.ts(j, SJ)
                aot = aotpool.tile([128, HPG, SJ], BF16)  # attn-out.T [dk, sq]
                for h in range(HPG):
                    attns = []
                    for grp in range(NG):
                        ps2 = psS.tile([128, GRP, SJ], F32)
                        for gi in range(GRP):
                            nc.tensor.matmul(
                                ps2[:, gi, :],
                                lhsT=kt_sb[:, bass.ts(GRP * grp + gi, 128)],
                                rhs=qt_sb[:, h, sl],
                                start=True, stop=True)
                        a = attnpool.tile([128, GRP, SJ], BF16)
                        nc.scalar.activation(
                            out=a, in_=ps2,
                            func=mybir.ActivationFunctionType.Exp, scale=SCALE)
                        attns.append(a)
                    for sub in range(SJ // 128):
                        pav = psAV.tile([128, PAV_W], F32, tag="av")
                        qsl = bass.ts(sub, 128)
                        for sk in range(NSK):
                            nc.tensor.matmul(
                                pav[:, 0:129],
                                lhsT=attns[sk // GRP][:, sk % GRP, qsl],
                                rhs=vones[:, sk, 0:129],
                                start=(sk == 0), stop=(sk == NSK - 1))
                        recip = smallpool.tile([128, 1], F32)
                        nc.vector.reciprocal(recip, pav[:, 128:129])
                        ao = smallpool.tile([128, 128], BF16, tag="ao")
                        nc.vector.tensor_scalar_mul(ao, pav[:, 0:128], recip)
                        pt = pav[:, 132:196].bitcast(BF16)
                        nc.tensor.transpose(pt, ao, ident16)
                        nc.vector.tensor_copy(aot[:, h, qsl], pt)
                # O projection for q rows [j*SJ, (j+1)*SJ)
                for sub in range(SJ // 128):
                    osb = outpool.tile([128, D], F32)
                    for dc in range(D // 512):
                        po = psO.tile([128, 512], F32, tag="o")
                        for h in range(HPG):
                            nc.tensor.matmul(
                                po, lhsT=aot[:, h, bass.ts(sub, 128)],
                                rhs=wo_sb[:, h, bass.ts(dc, 512)],
                                start=(h == 0), stop=(h == HPG - 1))
                        nc.vector.tensor_copy(osb[:, bass.ts(dc, 512)], po)
                    nc.sync.dma_start(
                        out=out[j * SJ + sub * 128: j * SJ + (sub + 1) * 128, :],
                        in_=osb)

    nc.compile()
    return nc


_NC_CACHE = None


def _get_program():
    global _NC_CACHE
    if _NC_CACHE is None:
        _NC_CACHE = build_program()
    return _NC_CACHE


def kernel(x, Wq, bq, Wk, bk, Wv, bv, Wo, bo):
    x = np.asarray(x, np.float32)
    nc = _get_program()

    bf = ml_dtypes.bfloat16
    in_maps = []
    xts = [np.ascontiguousarray(x[b].T).astype(bf) for b in range(x.shape[0])]
    Wq = np.asarray(Wq, np.float32)
    Wk = np.asarray(Wk, np.float32)
    Wv = np.asarray(Wv, np.float32)
    Wo = np.asarray(Wo, np.float32)
    for c in range(N_CORES):
        b, g = divmod(c, HPG)
        bv_g = np.asarray(bv, np.float32)[g * DK:(g + 1) * DK]
        in_maps.append({
            "xt": xts[b],
            "wq": np.ascontiguousarray(Wq[:, g * QCOLS:(g + 1) * QCOLS]).astype(bf),
            "wk": np.ascontiguousarray(Wk[:, g * DK:(g + 1) * DK]).astype(bf),
            "wv": np.ascontiguousarray(Wv[:, g * DK:(g + 1) * DK]).astype(bf),
            "wo": np.ascontiguousarray(Wo[g * QCOLS:(g + 1) * QCOLS, :]).astype(bf),
            "bq": np.ascontiguousarray(np.asarray(bq, np.float32)[g * QCOLS:(g + 1) * QCOLS]),
            "bk": np.ascontiguousarray(np.asarray(bk, np.float32)[g * DK:(g + 1) * DK]),
            "bvb": np.ascontiguousarray(np.tile(bv_g, (DK, 1))),
        })

    res = run_bass_kernel_spmd(nc, in_maps, core_ids=list(range(N_CORES))).results

    outv = np.zeros((x.shape[0], S, D), np.float32)
    for c in range(N_CORES):
        b = c // HPG
        outv[b] += res[c]["out"]
    outv += np.asarray(bo, np.float32)
    return outv


# revision 11
# speedup vs baseline: 1.4192x; 1.1287x over previous
"""GQA kernel for Trainium2, 8 NeuronCores.

Problem: x[2,2048,2048] -> GQA(16 heads, 4 kv groups, dk=128) -> out[2,2048,2048]

Sharding: core c handles (batch b = c//4, kv-group g = c%4), i.e. the 4 query
heads of one group on one batch. Zero replication of FLOPs across cores.
Host sums the 4 per-group partial outputs per batch (row-parallel O-proj
reduction) and adds bo.

All matmul operands are bf16 (host-converted), PSUM accumulation stays f32:
  xT [D,S] bf16 (host-transposed)  --PE-->  QT [dk,S]/head, KT [dk,S] (bf16)
  V is projected directly in [t, dk] orientation (lhsT = xT chunk), so no
    V transpose is needed; a ones-column next to V gives softmax denominators.
  scoresT[sk,sq] = KT_chunk.T @ QT   (bf16, 512-wide)
  exp on ACT in [128, 2x512] chunks from 2 PSUM banks (amortizes overhead);
    biases are applied on DVE so ACT does nothing but exp.
  attn@[V|1] accumulates over the 16 key chunks; DVE normalizes by the
    denominator column; PE transposes attn-out back to [dk, sq] (bf16);
    row-parallel O-projection accumulates the 4 heads into PSUM.
"""

import math

import numpy as np
import ml_dtypes

import concourse.bass as bass
import concourse.mybir as mybir
import concourse.tile as tile
from concourse import bacc
from concourse.bass_utils import run_bass_kernel_spmd
from concourse.masks import make_identity

F32 = mybir.dt.float32
BF16 = mybir.dt.bfloat16

D = 2048          # d_model
S = 2048          # seq len
DK = 128          # head dim
HPG = 4           # heads per kv group
QCOLS = HPG * DK  # 512 q columns per core
N_CORES = 8
SCALE = 1.0 / math.sqrt(DK)

SJ = 512                    # seq chunk (free dim of proj/scores matmuls)
NJ = S // SJ                # 4 chunks
NSK = S // 128              # 16 key chunks
ND = D // 128               # 16 d_model chunks
GRP = 2                     # key chunks per exp group (2 PSUM banks)
NG = NSK // GRP             # 8 exp groups
PAV_W = 196                 # AV psum tile width: [0:129]=AV|denom, f32[132:196]
                            # bitcast to bf16 [128] = transpose scratch


def build_program(n_reps=1):
    nc = bacc.Bacc("TRN2", target_bir_lowering=False, debug=False,
                   num_devices=N_CORES)

    xt = nc.dram_tensor("xt", [D, S], BF16, kind="ExternalInput").ap()
    wq = nc.dram_tensor("wq", [D, QCOLS], BF16, kind="ExternalInput").ap()
    wk = nc.dram_tensor("wk", [D, DK], BF16, kind="ExternalInput").ap()
    wv = nc.dram_tensor("wv", [D, DK], BF16, kind="ExternalInput").ap()
    wo = nc.dram_tensor("wo", [QCOLS, D], BF16, kind="ExternalInput").ap()
    bq = nc.dram_tensor("bq", [QCOLS], F32, kind="ExternalInput").ap()
    bk = nc.dram_tensor("bk", [DK], F32, kind="ExternalInput").ap()
    bvb = nc.dram_tensor("bvb", [DK, DK], F32, kind="ExternalInput").ap()
    out = nc.dram_tensor("out", [S, D], BF16, kind="ExternalOutput").ap()

    with tile.TileContext(nc) as tc:
      for _rep in range(n_reps):
        with (
            tc.tile_pool(name="singles", bufs=1) as singles,
            tc.tile_pool(name="xp", bufs=2) as xpool,
            tc.tile_pool(name="attn", bufs=18) as attnpool,
            tc.tile_pool(name="aot", bufs=2) as aotpool,
            tc.tile_pool(name="osb", bufs=3) as outpool,
            tc.tile_pool(name="small", bufs=6) as smallpool,
            tc.tile_pool(name="psS", bufs=2, space="PSUM") as psS,
            tc.tile_pool(name="psAV", bufs=2, space="PSUM") as psAV,
            tc.tile_pool(name="psO", bufs=2, space="PSUM") as psO,
        ):
            # ---- resident weights / biases; DMA order = lead-in order ----
            wk_sb = singles.tile([128, ND, DK], BF16)
            nc.sync.dma_start(out=wk_sb, in_=wk.rearrange("(c p) n -> p c n", p=128))
            bk_sb = singles.tile([128, 1], F32)
            nc.sync.dma_start(out=bk_sb, in_=bk.unsqueeze(1))
            wv_sb = singles.tile([128, ND, DK], BF16)
            nc.sync.dma_start(out=wv_sb, in_=wv.rearrange("(c p) n -> p c n", p=128))
            bvb_sb = singles.tile([128, DK], F32)
            nc.sync.dma_start(out=bvb_sb, in_=bvb)

            ident16 = singles.tile([128, 128], BF16)
            make_identity(nc, ident16)

            qt_sb = singles.tile([128, HPG, S], BF16)    # QT per head [dk, S]
            kt_sb = singles.tile([128, S], BF16)         # KT [dk, S]
            vones = singles.tile([128, NSK, 132], BF16)  # [V | 1] per key chunk
            nc.vector.memset(vones[:, :, 128:129], 1.0)

            wq_sb = singles.tile([128, ND, QCOLS], BF16)
            bq_sb = singles.tile([128, HPG], F32)
            wo_sb = singles.tile([128, HPG, D], BF16)

            # ---- phase B: projections (stream xT chunks) ----
            for j in range(NJ):
                sl = bass.ts(j, SJ)
                xt_sb = xpool.tile([128, ND, SJ], BF16)
                xsrc = xt[:, sl].rearrange("(c p) s -> p c s", p=128)
                for q in range(4):
                    nc.sync.dma_start(out=xt_sb[:, 4 * q:4 * q + 4, :],
                                      in_=xsrc[:, 4 * q:4 * q + 4, :])
                if j == 0:
                    # late weights stream behind the first x chunk
                    nc.sync.dma_start(
                        out=wq_sb, in_=wq.rearrange("(c p) n -> p c n", p=128))
                    nc.sync.dma_start(
                        out=bq_sb, in_=bq.rearrange("(h p) -> p h", p=128))
                    nc.sync.dma_start(
                        out=wo_sb, in_=wo.rearrange("(h p) n -> p h n", p=128))
                pk = psO.tile([128, SJ], F32, tag="o")
                for d in range(ND):
                    nc.tensor.matmul(pk, lhsT=wk_sb[:, d, :],
                                     rhs=xt_sb[:, d, :],
                                     start=(d == 0), stop=(d == ND - 1))
                nc.vector.tensor_scalar_add(kt_sb[:, sl], pk, bk_sb)
                for sub in range(SJ // 128):
                    pv = psAV.tile([128, PAV_W], F32, tag="av")
                    tsl = bass.ts(sub, 128)
                    for d in range(ND):
                        nc.tensor.matmul(pv[:, 0:128], lhsT=xt_sb[:, d, tsl],
                                         rhs=wv_sb[:, d, :],
                                         start=(d == 0), stop=(d == ND - 1))
                    nc.vector.tensor_add(vones[:, 4 * j + sub, 0:128],
                                         pv[:, 0:128], bvb_sb)
                for h in range(HPG):
                    pq = psO.tile([128, SJ], F32, tag="o")
                    for d in range(ND):
                        nc.tensor.matmul(
                            pq, lhsT=wq_sb[:, d, bass.ts(h, 128)],
                            rhs=xt_sb[:, d, :],
                            start=(d == 0), stop=(d == ND - 1))
                    nc.vector.tensor_scalar_add(qt_sb[:, h, sl], pq,
                                                bq_sb[:, h:h + 1])

            # ---- phase C: attention + O-projection, software-pipelined ----
            # Slot s = (j, h). Each loop iteration emits, interleaved in the
            # PE stream: the scores+exp groups of slot s, the AV/normalize/
            # transpose sub-units of slot s-1, and one quarter of the O
            # projection of q-block j-1 (whose aot completed 1..4 slots ago).
            # The interleave keeps the in-order PE queue from head-blocking
            # on ACT's exp latency, and drains the O matmuls + out DMA while
            # the next block's attention runs.
            slots = [(j, h) for j in range(NJ) for h in range(HPG)]
            aot_tiles = {}
            prev = None          # (j, h, attns) of slot s-1
            pending_o = []       # deferred O-proj (j, sub, dc) units
            osb_tiles = {}

            def emit_scores_group(j, h, grp, attns):
                ps2 = psS.tile([128, GRP, SJ], F32)
                for gi in range(GRP):
                    nc.tensor.matmul(
                        ps2[:, gi, :],
                        lhsT=kt_sb[:, bass.ts(GRP * grp + gi, 128)],
                        rhs=qt_sb[:, h, bass.ts(j, SJ)],
                        start=True, stop=True)
                a = attnpool.tile([128, GRP, SJ], BF16)
                nc.scalar.activation(
                    out=a, in_=ps2,
                    func=mybir.ActivationFunctionType.Exp, scale=SCALE)
                attns.append(a)

            def emit_av_unit(j, h, sub, attns):
                pav = psAV.tile([128, PAV_W], F32, tag="av")
                qsl = bass.ts(sub, 128)
                for sk in range(NSK):
                    nc.tensor.matmul(
                        pav[:, 0:129],
                        lhsT=attns[sk // GRP][:, sk % GRP, qsl],
                        rhs=vones[:, sk, 0:129],
                        start=(sk == 0), stop=(sk == NSK - 1))
                recip = smallpool.tile([128, 1], F32)
                nc.vector.reciprocal(recip, pav[:, 128:129])
                ao = smallpool.tile([128, 128], BF16, tag="ao")
                nc.vector.tensor_scalar_mul(ao, pav[:, 0:128], recip)
                pt = pav[:, 132:196].bitcast(BF16)
                nc.tensor.transpose(pt, ao, ident16)
                nc.vector.tensor_copy(aot_tiles[j][:, h, qsl], pt)

            def emit_o_unit(j, sub, dc):
                if dc == 0:
                    osb_tiles[(j, sub)] = outpool.tile([128, D], BF16,
                                                       name="osb")
                osb = osb_tiles[(j, sub)]
                po = psO.tile([128, 512], F32, tag="o")
                for h in range(HPG):
                    nc.tensor.matmul(
                        po, lhsT=aot_tiles[j][:, h, bass.ts(sub, 128)],
                        rhs=wo_sb[:, h, bass.ts(dc, 512)],
                        start=(h == 0), stop=(h == HPG - 1))
                nc.vector.tensor_copy(osb[:, bass.ts(dc, 512)], po)
                if dc == D // 512 - 1:
                    nc.sync.dma_start(
                        out=out[j * SJ + sub * 128: j * SJ + (sub + 1) * 128, :],
                        in_=osb)
                    del osb_tiles[(j, sub)]

            for s in range(len(slots) + 1):
                units = []
                if prev is not None:
                    pj, ph, pattns = prev
                    units += [("av", pj, ph, sub, pattns)
                              for sub in range(SJ // 128)]
                units += pending_o[:4]
                pending_o = pending_o[4:]

                if s < len(slots):
                    j, h = slots[s]
                    if h == 0:
                        aot_tiles[j] = aotpool.tile([128, HPG, SJ], BF16,
                                                    name="aot")
                    attns = []
                    # g0 g1 u0 g2 g3 u1 g4 g5 u2 g6 g7 u3 <rest of units>
                    ui = 0
                    for grp in range(NG):
                        emit_scores_group(j, h, grp, attns)
                        if grp % 2 == 1 and ui < len(units):
                            u = units[ui]
                            ui += 1
                            if u[0] == "av":
                                emit_av_unit(*u[1:])
                            else:
                                emit_o_unit(*u[1:])
                    units = units[ui:]
                for u in units:
                    if u[0] == "av":
                        emit_av_unit(*u[1:])
                    else:
                        emit_o_unit(*u[1:])
                if prev is not None and prev[1] == HPG - 1:
                    pending_o += [("o", prev[0], sub, dc)
                                  for sub in range(SJ // 128)
                                  for dc in range(D // 512)]
                prev = (slots[s][0], slots[s][1], attns) if s < len(slots) else None
            for u in pending_o:
                emit_o_unit(*u[1:])

    nc.compile()
    return nc


_NC_CACHE = None


def _get_program():
    global _NC_CACHE
    if _NC_CACHE is None:
        _NC_CACHE = build_program()
    return _NC_CACHE


def kernel(x, Wq, bq, Wk, bk, Wv, bv, Wo, bo):
    x = np.asarray(x, np.float32)
    nc = _get_program()

    bf = ml_dtypes.bfloat16
    in_maps = []
    xts = [np.ascontiguousarray(x[b].T).astype(bf) for b in range(x.shape[0])]
    Wq = np.asarray(Wq, np.float32)
    Wk = np.asarray(Wk, np.float32)
    Wv = np.asarray(Wv, np.float32)
    Wo = np.asarray(Wo, np.float32)
    for c in range(N_CORES):
        b, g = divmod(c, HPG)
        bv_g = np.asarray(bv, np.float32)[g * DK:(g + 1) * DK]
        in_maps.append({
            "xt": xts[b],
            "wq": np.ascontiguousarray(Wq[:, g * QCOLS:(g + 1) * QCOLS]).astype(bf),
            "wk": np.ascontiguousarray(Wk[:, g * DK:(g + 1) * DK]).astype(bf),
            "wv": np.ascontiguousarray(Wv[:, g * DK:(g + 1) * DK]).astype(bf),
            "wo": np.ascontiguousarray(Wo[g * QCOLS:(g + 1) * QCOLS, :]).astype(bf),
            "bq": np.ascontiguousarray(np.asarray(bq, np.float32)[g * QCOLS:(g + 1) * QCOLS]),
            "bk": np.ascontiguousarray(np.asarray(bk, np.float32)[g * DK:(g + 1) * DK]),
            "bvb": np.ascontiguousarray(np.tile(bv_g, (DK, 1))),
        })

    res = run_bass_kernel_spmd(nc, in_maps, core_ids=list(range(N_CORES))).results

    outv = np.zeros((x.shape[0], S, D), np.float32)
    for c in range(N_CORES):
        b = c // HPG
        outv[b] += res[c]["out"]
    outv += np.asarray(bo, np.float32)
    return outv
